# revision 5
# baseline (speedup 1.0000x reference)
"""Trainium2 Bass kernel for nn_DCSRM — bf16, gutter layout, v3.

Math: conv(x*g, w) = conv(x, w*g); stats -> g -> dynamic 3x3 taps folded
with g; depthwise conv as 9 shifted accumulations.

v3 layout: each image row padded 96->98 with two zero gutter columns, so
every shifted flat read lands on zeros at row edges — no wrap fixups.
PE does 80 rows/tile as 5-row x 488-col psum chunks (taps outer over 6
banks, walrus LDW dedup enabled); DVE does 16 rows/tile via 8-tap STT
(center tap on ScalarE).  Stats: (0,0) DVE bn_stats, (0,1)+(1,1) ScalarE
Copy/Square+accum, (1,0) GpSimd square/reduce.  ScalarE drains psum to
bf16 stages; ACT tables preloaded and PE HAM-warmed during the DMA head.
"""
import os
import sys
import types
import contextlib
from contextlib import ExitStack

sys.path.insert(0, '/opt/trn_rl_repo')

import numpy as np
import ml_dtypes

BF = np.dtype(ml_dtypes.bfloat16)

N, C, H, W = 16, 256, 96, 96
EPS = 1e-5
NPIX = H * W
CORES = 8
S_PER_CORE = N // CORES

W2 = 98                            # padded row width (2 zero gutters)
GUARD_TOP = 2
GUARD_BOT = 3
ROWS_BUF = GUARD_TOP + H + GUARD_BOT           # 101
XT_LEN = ROWS_BUF * W2                          # 9898
DATA_OFF = GUARD_TOP * W2                       # 196

P_PE, P_DVE = 80, 16               # conv row split per tile
PE_CHUNK = 5                       # rows per psum chunk (5*98=490 f32)
PE_BANKS = 6

LAST_EXEC_NS = None
LAST_RESULTS = None
_PROGRAM_CACHE = {}


def _install_trace_hook_shim():
    try:
        import antenv.axon_hooks  # noqa: F401
        return
    except ImportError:
        pass
    try:
        import antenv
        import ctypes
    except ImportError:
        return
    so_path = '/opt/axon/libaxon_pjrt.so'

    def _build():
        if not os.path.exists(so_path):
            return None
        lib = ctypes.CDLL(so_path)
        if not hasattr(lib, 'axon_start_nrt_profile'):
            return None
        lib.axon_start_nrt_profile.argtypes = [
            ctypes.POINTER(ctypes.c_int64), ctypes.c_size_t]
        lib.axon_start_nrt_profile.restype = ctypes.c_int64
        lib.axon_stop_nrt_profile.argtypes = [ctypes.c_char_p]
        lib.axon_stop_nrt_profile.restype = ctypes.c_int64

        @contextlib.contextmanager
        def _hook(output_dir, device_ids):
            import jax
            jax.devices()
            if device_ids:
                ids = (ctypes.c_int64 * len(device_ids))(*device_ids)
                rc = lib.axon_start_nrt_profile(ids, len(device_ids))
            else:
                rc = lib.axon_start_nrt_profile(None, 0)
            if rc != 0:
                raise RuntimeError(f'axon_start_nrt_profile rc={rc}')
            try:
                yield
            finally:
                n = lib.axon_stop_nrt_profile(str(output_dir).encode())
                print(f'ntff profile: {n} file(s) -> {output_dir}',
                      file=sys.stderr)
        return _hook

    mod = types.ModuleType('antenv.axon_hooks')
    holder = {'hook': _build()}
    mod.get_axon_ntff_profile_hook = lambda: holder['hook']
    mod.set_axon_ntff_profile_hook = lambda h: holder.update(hook=h)
    sys.modules['antenv.axon_hooks'] = mod
    antenv.axon_hooks = mod


def _enable_ldw_opt():
    """Let walrus dedupe consecutive identical LDWEIGHTS (off by default).
    Correctness is covered by the caller's rel-err check."""
    from concourse import bass_utils
    if getattr(bass_utils, '_ldw_patched', False):
        return
    orig = bass_utils.run_command

    def run_command(cmd, **kw):
        cmd = ['--enable-ldw-opt=true' if c == '--enable-ldw-opt=false'
               else c for c in cmd]
        return orig(cmd, **kw)

    bass_utils.run_command = run_command
    bass_utils._ldw_patched = True


def _tap(k):
    return k // 3 - 1, k % 3 - 1


def _build_program():
    from concourse import bacc, mybir, tile

    F32 = mybir.dt.float32
    BF16 = mybir.dt.bfloat16
    OP = mybir.AluOpType
    AF = mybir.ActivationFunctionType

    nc = bacc.Bacc('TRN2', target_bir_lowering=False, debug=False,
                   num_devices=CORES)

    x_d = nc.dram_tensor('x', [S_PER_CORE, C, H, W], BF16,
                         kind='ExternalInput').ap()
    ca_d = nc.dram_tensor('constA', [128, 183], F32,
                          kind='ExternalInput').ap()
    cb_d = nc.dram_tensor('constB', [16, 2305], F32,
                          kind='ExternalInput').ap()
    out_d = nc.dram_tensor('out', [S_PER_CORE, C, H, W], BF16,
                           kind='ExternalOutput').ap()

    assert P_PE + P_DVE == H and P_PE % PE_CHUNK == 0
    NCH = P_PE // PE_CHUNK                      # 16 chunks per tile

    with tile.TileContext(nc) as tc:
        with ExitStack() as ctx:
            cpool = ctx.enter_context(tc.tile_pool(name='const', bufs=1))
            xpool = ctx.enter_context(tc.tile_pool(name='x', bufs=4))
            bnpool = ctx.enter_context(tc.tile_pool(name='bn', bufs=4))
            dumppool = ctx.enter_context(tc.tile_pool(name='dump', bufs=1))
            smpool = ctx.enter_context(tc.tile_pool(name='small', bufs=4))
            dgpool = ctx.enter_context(tc.tile_pool(name='diag', bufs=2))
            stpe = ctx.enter_context(tc.tile_pool(name='stage_pe', bufs=3))
            stdve = ctx.enter_context(tc.tile_pool(name='stage_dve',
                                                   bufs=2))
            pspool = ctx.enter_context(
                tc.tile_pool(name='psum', bufs=PE_BANKS, space='PSUM'))
            pssm = ctx.enter_context(
                tc.tile_pool(name='psum_s', bufs=1, space='PSUM'))

            constA = cpool.tile([128, 183], F32)
            constB = cpool.tile([16, 2305], F32)
            zsc = cpool.tile([128, 488], BF16)     # zero scratch

            # ---- t=0: act-table preload + PE HAM warmup (no DMA deps)
            nc.gpsimd.memset(zsc[:], 0)
            tpre = smpool.tile([128, 1], F32, tag='tpre', name='tpre')
            for af in (AF.Copy, AF.Square, AF.Sqrt, AF.Sigmoid, AF.Relu):
                nc.scalar.activation(tpre[:], zsc[:, 0:1], af)
            wps = pssm.tile([128, 488], F32, tag='dyn', name='warmps')
            for i in range(40):
                nc.tensor.matmul(wps[:], lhsT=zsc[:, 0:128],
                                 rhs=zsc[:],
                                 start=(i == 0), stop=(i == 39),
                                 skip_group_check=True)

            nc.sync.dma_start(constA[:], ca_d[:])
            nc.sync.dma_start(constB[:], cb_d[:])

            def load_tile(s, h, emit_dma=True):
                xt = xpool.tile([128, XT_LEN], BF16, tag='x',
                                name=f'x{s}{h}')
                xtv = xt[:].rearrange('p (r c) -> p r c', c=W2)
                nc.gpsimd.memset(xt[:, 0:DATA_OFF], 0)
                nc.gpsimd.memset(
                    xt[:, DATA_OFF + H * W2:XT_LEN], 0)
                nc.gpsimd.memset(xtv[:, GUARD_TOP:GUARD_TOP + H, W:W2], 0)
                src = x_d[s, 128 * h:128 * (h + 1)]
                dmas = []
                for q in range(4):
                    r0 = 24 * q

                    def dma(q=q, r0=r0):
                        nc.sync.dma_start(
                            xtv[:, GUARD_TOP + r0:GUARD_TOP + r0 + 24,
                                0:W],
                            src[:, r0:r0 + 24, :])
                    if emit_dma:
                        dma()
                    else:
                        dmas.append(dma)
                return (xt, xtv, dmas)

            def stats_dve(xt, tag):
                # groups span the zero gutters; correct for padded count
                PADN = H * W2
                bn6 = bnpool.tile([128, 24, 6], F32, tag='bn6',
                                  name=f'bn6{tag}')
                for g in range(24):
                    lo = DATA_OFF + 392 * g
                    nc.vector.bn_stats(bn6[:, g, :], xt[:, lo:lo + 392])
                mvp = bnpool.tile([128, 2], F32, tag='mvp',
                                  name=f'mvp{tag}')
                nc.vector.bn_aggr(mvp[:], bn6[:])
                sq = bnpool.tile([128, 2], F32, tag='sq',
                                 name=f'sqd{tag}')
                nc.vector.tensor_scalar_mul(sq[:, 0:1], mvp[:, 0:1],
                                            float(PADN))
                nc.vector.scalar_tensor_tensor(
                    sq[:, 1:2], mvp[:, 0:1], mvp[:, 0:1], mvp[:, 1:2],
                    OP.mult, OP.add)
                nc.vector.tensor_scalar_mul(sq[:, 1:2], sq[:, 1:2],
                                            float(PADN))
                return _moments(sq, tag)

            def stats_scalar(xtv, tag):
                sacc = bnpool.tile([128, 8], F32, tag='sacc',
                                   name=f'sacc{tag}')
                qacc = bnpool.tile([128, 8], F32, tag='qacc',
                                   name=f'qacc{tag}')
                ops = []
                for q in range(8):
                    r0 = 12 * q

                    def op_copy(q=q, r0=r0):
                        dump = dumppool.tile([128, 12 * W], BF16,
                                             tag='dump', name='dump')
                        nc.scalar.activation(
                            dump[:].rearrange('p (r c) -> p r c', c=W),
                            xtv[:, GUARD_TOP + r0:GUARD_TOP + r0 + 12,
                                0:W],
                            AF.Copy, accum_out=sacc[:, q:q + 1])

                    def op_sq(q=q, r0=r0):
                        dump = dumppool.tile([128, 12 * W], BF16,
                                             tag='dump', name='dump')
                        nc.scalar.activation(
                            dump[:].rearrange('p (r c) -> p r c', c=W),
                            xtv[:, GUARD_TOP + r0:GUARD_TOP + r0 + 12,
                                0:W],
                            AF.Square, accum_out=qacc[:, q:q + 1])

                    ops += [op_copy, op_sq]

                def finalize():
                    sq = bnpool.tile([128, 2], F32, tag='sq',
                                     name=f'sq{tag}')
                    nc.vector.tensor_reduce(sq[:, 0:1], sacc[:],
                                            axis=mybir.AxisListType.X,
                                            op=OP.add)
                    nc.vector.tensor_reduce(sq[:, 1:2], qacc[:],
                                            axis=mybir.AxisListType.X,
                                            op=OP.add)
                    return _moments(sq, tag)

                return ops, finalize

            def _moments(sq, tag):
                """sq[:,0]=sum, sq[:,1]=sumsq -> mv (mean, pop var)."""
                mv = bnpool.tile([128, 2], F32, tag='mv',
                                 name=f'mvz{tag}')
                mom = bnpool.tile([128, 2], F32, tag='mom',
                                  name=f'mom{tag}')
                nc.vector.tensor_scalar_mul(mom[:], sq[:], 1.0 / NPIX)
                nc.vector.tensor_copy(mv[:, 0:1], mom[:, 0:1])
                nc.vector.scalar_tensor_tensor(
                    mv[:, 1:2], mom[:, 0:1], mom[:, 0:1], mom[:, 1:2],
                    OP.mult, OP.subtract)
                nc.vector.tensor_scalar_mul(mv[:, 1:2], mv[:, 1:2], -1.0)
                return mv

            def sample_weights(s, mvs):
                g_cols = {}
                gap = smpool.tile([128, 2], F32, tag='gap', name=f'gap{s}')
                for h in (0, 1):
                    m = mvs[h]
                    mean = m[:, 0:1]
                    std = smpool.tile([128, 1], F32, tag='std',
                                      name=f'std{s}{h}')
                    nc.scalar.activation(std[:], m[:, 1:2], AF.Sqrt,
                                         bias=constA[:, 182:183],
                                         scale=float(NPIX) / (NPIX - 1))
                    zt = smpool.tile([128, 1], F32, tag='zt',
                                     name=f'zt{s}{h}')
                    nc.vector.tensor_tensor(zt[:], std[:],
                                            constA[:, 162 + h:163 + h],
                                            OP.mult)
                    nc.vector.scalar_tensor_tensor(
                        zt[:], mean, constA[:, 160 + h:161 + h], zt[:],
                        OP.mult, OP.add)
                    g = smpool.tile([128, 1], F32, tag='g', name=f'g{s}{h}')
                    nc.scalar.activation(g[:], zt[:], AF.Sigmoid)
                    g_cols[h] = g
                    nc.vector.tensor_tensor(gap[:, h:h + 1], g[:], mean,
                                            OP.mult)
                p = pssm.tile([16, 2], F32, tag='hid', name=f'hid{s}')
                for h in (0, 1):
                    nc.tensor.matmul(p[:, h:h + 1],
                                     lhsT=constA[:, 128 + 16 * h:
                                                 144 + 16 * h],
                                     rhs=gap[:, h:h + 1],
                                     start=True, stop=True)
                hsum = smpool.tile([16, 1], F32, tag='hsum',
                                   name=f'hsum{s}')
                nc.vector.tensor_reduce(hsum[:], p[:],
                                        axis=mybir.AxisListType.X,
                                        op=OP.add)
                hid = smpool.tile([16, 1], F32, tag='hid_sb',
                                  name=f'hidsb{s}')
                nc.scalar.activation(hid[:], hsum[:], AF.Relu,
                                     bias=constB[:, 2304:2305], scale=1.0)
                per_half = {}
                for h in (0, 1):
                    pd = pssm.tile([128, 9], F32, tag='dyn',
                                   name=f'dyn{s}{h}')
                    for k in range(9):
                        nc.tensor.matmul(
                            pd[:, k:k + 1],
                            lhsT=constB[:, k * 256 + 128 * h:
                                        k * 256 + 128 * h + 128],
                            rhs=hid[:], start=True, stop=True)
                    b2g = smpool.tile([128, 9], F32, tag='b2g',
                                      name=f'b2g{s}{h}')
                    nc.vector.tensor_scalar_mul(
                        b2g[:], constA[:, 164 + 9 * h:173 + 9 * h],
                        g_cols[h][:])
                    weff = smpool.tile([128, 9], F32, tag='weff',
                                       name=f'weff{s}{h}')
                    nc.vector.scalar_tensor_tensor(
                        weff[:], pd[:], g_cols[h][:], b2g[:], OP.mult,
                        OP.add)
                    weff16 = smpool.tile([128, 9], BF16, tag='weff16',
                                         name=f'weff16{s}{h}')
                    nc.vector.tensor_copy(weff16[:], weff[:])
                    dg = dgpool.tile([128, 9 * 128], BF16, tag='diag',
                                     name=f'diag{s}{h}')
                    for k in range(9):
                        nc.scalar.activation(
                            dg[:, 128 * k:128 * (k + 1)],
                            constA[:, 0:128], AF.Copy,
                            scale=weff[:, k:k + 1])
                    per_half[h] = (weff, weff16, dg)
                return per_half

            def conv_tile(s, h, xt, xtv, weff, weff16, dg, fillers=None,
                          last=False):
                fillers = list(fillers or [])
                out_flat = out_d[s, 128 * h:128 * (h + 1)].rearrange(
                    'c a b -> c (a b)')

                def take_fillers(k):
                    for _ in range(min(k, len(fillers))):
                        fillers.pop(0)()

                # PE rows [0, P_PE) in rounds of PE_BANKS 5-row chunks
                rs = 0
                while rs < P_PE:
                    re = min(rs + PE_BANKS * PE_CHUNK, P_PE)
                    nchunk = (re - rs) // PE_CHUNK
                    ps = [pspool.tile([128, 490], F32, tag='cps',
                                      name=f'cps{s}{h}{rs}_{b}')
                          for b in range(nchunk)]
                    for k in range(9):
                        dy, dx = _tap(k)
                        for b in range(nchunk):
                            rc = rs + b * PE_CHUNK
                            off = (GUARD_TOP + rc + dy) * W2 + dx
                            nc.tensor.matmul(
                                ps[b][:, 0:488],
                                lhsT=dg[:, 128 * k:128 * (k + 1)],
                                rhs=xt[:, off:off + 488],
                                start=(k == 0), stop=(k == 8),
                                skip_group_check=True)
                    st = stpe.tile([128, (re - rs) * W], BF16,
                                   tag='st_pe', name=f'stpe{s}{h}{rs}')
                    stv = st[:].rearrange('p (r c) -> p r c', c=W)
                    for b in range(nchunk):
                        psv = ps[b][:].rearrange('p (r c) -> p r c',
                                                 c=W2)
                        nc.scalar.copy(
                            stv[:, b * PE_CHUNK:(b + 1) * PE_CHUNK, :],
                            psv[:, :, 0:W])
                        if b % 2 == 0:
                            take_fillers(1)
                    nc.sync.dma_start(out_flat[:, rs * W:re * W], st[:])
                    rs = re

                # DVE rows [P_PE, H): center tap ACT, 8 STT taps
                a, b_ = P_PE, H
                nrow = b_ - a
                st = stdve.tile([128, nrow * W], BF16, tag='st_dve',
                                name=f'stdve{s}{h}')
                stv = st[:].rearrange('p (r c) -> p r c', c=W)
                for k in (4, 0, 1, 2, 3, 5, 6, 7, 8):
                    dy, dx = _tap(k)
                    base = (GUARD_TOP + a + dy) * W2 + dx
                    i_ap = xt[:, base:base + nrow * W2].rearrange(
                        'p (r c) -> p r c', c=W2)[:, :, 0:W]
                    if k == 4:
                        nc.scalar.activation(stv[:], i_ap, AF.Copy,
                                             scale=weff[:, 4:5])
                        take_fillers(1)
                    else:
                        nc.vector.scalar_tensor_tensor(
                            stv[:], i_ap, weff16[:, k:k + 1], stv[:],
                            OP.mult, OP.add)
                if last:
                    mid = a + nrow // 2
                    nc.sync.dma_start(out_flat[:, a * W:mid * W],
                                      st[:, 0:(mid - a) * W])
                    nc.sync.dma_start(out_flat[:, mid * W:b_ * W],
                                      st[:, (mid - a) * W:])
                else:
                    nc.sync.dma_start(out_flat[:, a * W:b_ * W], st[:])
                for f in fillers:
                    f()

            # ================= pipeline =================
            xt00, xv00, d00 = load_tile(0, 0, emit_dma=False)
            xt01, xv01, d01 = load_tile(0, 1, emit_dma=False)
            for q in range(4):                 # interleave sample-0 loads
                d00[q]()
                d01[q]()
            xt10, xv10, _ = load_tile(1, 0)
            xt11, xv11, _ = load_tile(1, 1)
            mv00 = stats_dve(xt00, '00')
            ops01, fin01 = stats_scalar(xv01, '01')
            for op in ops01:
                op()
            mv01 = fin01()
            ph0 = sample_weights(0, {0: mv00, 1: mv01})
            ops11, fin11 = stats_scalar(xv11, '11')
            conv_tile(0, 0, xt00, xv00, *ph0[0], fillers=ops11)
            mv10 = stats_dve(xt10, '10')
            conv_tile(0, 1, xt01, xv01, *ph0[1])
            mv11 = fin11()
            ph1 = sample_weights(1, {0: mv10, 1: mv11})
            conv_tile(1, 0, xt10, xv10, *ph1[0])
            conv_tile(1, 1, xt11, xv11, *ph1[1], last=True)

    nc.compile()
    return nc


def _host_constants(cfc, w1, b1, w2, b2):
    A = np.zeros((128, 183), np.float32)
    A[:, 0:128] = np.eye(128, dtype=np.float32)
    w1T = np.ascontiguousarray(w1.T)
    A[:, 128:144] = w1T[:128]
    A[:, 144:160] = w1T[128:]
    A[:, 160] = cfc[0:128, 0]
    A[:, 161] = cfc[128:256, 0]
    A[:, 162] = cfc[0:128, 1]
    A[:, 163] = cfc[128:256, 1]
    b2r = b2.reshape(256, 9)
    A[:, 164:173] = b2r[0:128]
    A[:, 173:182] = b2r[128:256]
    A[:, 182] = EPS
    w2p = w2.reshape(256, 9, 16).transpose(1, 0, 2).reshape(2304, 16)
    B = np.zeros((16, 2305), np.float32)
    B[:, 0:2304] = w2p.T
    B[:, 2304] = b1
    return A, B


def kernel(x, cfc, w1, b1, w2, b2):
    global LAST_EXEC_NS, LAST_RESULTS
    _install_trace_hook_shim()
    from concourse.bass_utils import run_bass_kernel_spmd

    x16 = np.ascontiguousarray(np.asarray(x, np.float32)).astype(BF)
    A, B = _host_constants(np.asarray(cfc, np.float32),
                           np.asarray(w1, np.float32),
                           np.asarray(b1, np.float32),
                           np.asarray(w2, np.float32),
                           np.asarray(b2, np.float32))

    if 'nc' not in _PROGRAM_CACHE:
        _PROGRAM_CACHE['nc'] = _build_program()
    nc = _PROGRAM_CACHE['nc']

    in_maps = [{'x': x16[S_PER_CORE * i:S_PER_CORE * (i + 1)],
                'constA': A, 'constB': B} for i in range(CORES)]
    res = run_bass_kernel_spmd(nc, in_maps, list(range(CORES)))
    LAST_EXEC_NS = res.exec_time_ns
    LAST_RESULTS = res
    out = np.concatenate([res.results[i]['out'] for i in range(CORES)],
                         axis=0)
    return out.astype(np.float32)


# revision 6
# speedup vs baseline: 1.0424x; 1.0424x over previous
"""Trainium2 Bass kernel for nn_DCSRM — bf16 rewrite.

Same math as v1 (g folds into the 9 tap weights; srm never materialized)
but all image traffic is bf16: host converts x fp32->bf16, kernel writes
bf16 output, host converts back. Halves HBM bytes both ways.

Per core (2 samples x 2 channel-halves = 4 tiles [128ch, 99*96] bf16):
  stats:  DVE bn_stats for tiles h=0, ScalarE Copy/Square+accum for h=1
  conv:   rows split PE (diag-bf16 matmuls, taps-outer over 6 psum banks)
          / DVE (8-tap STT FMA, center tap on ScalarE)
          / GpSimd (8-tap STT FMA)
  drains: psum -> bf16 stage on ScalarE/GpSimd, wrap fixups on DVE
"""
import os
import sys
import types
import contextlib
from contextlib import ExitStack

sys.path.insert(0, '/opt/trn_rl_repo')

import numpy as np
import ml_dtypes

BF = np.dtype(ml_dtypes.bfloat16)

N, C, H, W = 16, 256, 96, 96
EPS = 1e-5
NPIX = H * W                      # 9216
CORES = 8
S_PER_CORE = N // CORES           # 2 samples per core

GUARD_TOP = 2
GUARD_BOT = 1
ROWS_BUF = GUARD_TOP + H + GUARD_BOT          # 99
XT_LEN = ROWS_BUF * W                          # 9504
DATA_OFF = GUARD_TOP * W                       # 192

# conv row ownership per tile: (pe_rows, dve_rows, gp_rows), sum = 96
ROW_SPLIT = (68, 28, 0)
PE_CHUNK = 4                      # rows per psum bank (4*96=384 <= 512)
PE_BANKS = 6                      # conv psum banks; 2 left for weights

LAST_EXEC_NS = None
LAST_RESULTS = None
_PROGRAM_CACHE = {}


def _install_trace_hook_shim():
    """Register the NTFF hook via ctypes if antenv.axon_hooks is absent."""
    try:
        import antenv.axon_hooks  # noqa: F401
        return
    except ImportError:
        pass
    try:
        import antenv
        import ctypes
    except ImportError:
        return
    so_path = '/opt/axon/libaxon_pjrt.so'

    def _build():
        if not os.path.exists(so_path):
            return None
        lib = ctypes.CDLL(so_path)
        if not hasattr(lib, 'axon_start_nrt_profile'):
            return None
        lib.axon_start_nrt_profile.argtypes = [
            ctypes.POINTER(ctypes.c_int64), ctypes.c_size_t]
        lib.axon_start_nrt_profile.restype = ctypes.c_int64
        lib.axon_stop_nrt_profile.argtypes = [ctypes.c_char_p]
        lib.axon_stop_nrt_profile.restype = ctypes.c_int64

        @contextlib.contextmanager
        def _hook(output_dir, device_ids):
            import jax
            jax.devices()
            if device_ids:
                ids = (ctypes.c_int64 * len(device_ids))(*device_ids)
                rc = lib.axon_start_nrt_profile(ids, len(device_ids))
            else:
                rc = lib.axon_start_nrt_profile(None, 0)
            if rc != 0:
                raise RuntimeError(f'axon_start_nrt_profile rc={rc}')
            try:
                yield
            finally:
                n = lib.axon_stop_nrt_profile(str(output_dir).encode())
                print(f'ntff profile: {n} file(s) -> {output_dir}',
                      file=sys.stderr)
        return _hook

    mod = types.ModuleType('antenv.axon_hooks')
    holder = {'hook': _build()}
    mod.get_axon_ntff_profile_hook = lambda: holder['hook']
    mod.set_axon_ntff_profile_hook = lambda h: holder.update(hook=h)
    sys.modules['antenv.axon_hooks'] = mod
    antenv.axon_hooks = mod


def _tap(k):
    return k // 3 - 1, k % 3 - 1          # dy, dx


def _build_program():
    from concourse import bacc, mybir, tile

    F32 = mybir.dt.float32
    BF16 = mybir.dt.bfloat16
    OP = mybir.AluOpType
    AF = mybir.ActivationFunctionType

    nc = bacc.Bacc('TRN2', target_bir_lowering=False, debug=False,
                   num_devices=CORES)

    x_d = nc.dram_tensor('x', [S_PER_CORE, C, H, W], BF16,
                         kind='ExternalInput').ap()
    ca_d = nc.dram_tensor('constA', [128, 183], F32,
                          kind='ExternalInput').ap()
    cb_d = nc.dram_tensor('constB', [16, 2305], F32,
                          kind='ExternalInput').ap()
    out_d = nc.dram_tensor('out', [S_PER_CORE, C, H, W], BF16,
                           kind='ExternalOutput').ap()

    P_PE, P_DVE, P_GP = ROW_SPLIT
    assert P_PE + P_DVE + P_GP == H
    assert P_PE % PE_CHUNK == 0

    with tile.TileContext(nc) as tc:
        with ExitStack() as ctx:
            cpool = ctx.enter_context(tc.tile_pool(name='const', bufs=1))
            xpool = ctx.enter_context(tc.tile_pool(name='x', bufs=4))
            bnpool = ctx.enter_context(tc.tile_pool(name='bn', bufs=4))
            dumppool = ctx.enter_context(tc.tile_pool(name='dump', bufs=1))
            smpool = ctx.enter_context(tc.tile_pool(name='small', bufs=4))
            dgpool = ctx.enter_context(tc.tile_pool(name='diag', bufs=2))
            stpe = ctx.enter_context(tc.tile_pool(name='stage_pe', bufs=3))
            stdve = ctx.enter_context(tc.tile_pool(name='stage_dve', bufs=2))
            stgp = ctx.enter_context(tc.tile_pool(name='stage_gp', bufs=2))
            pspool = ctx.enter_context(
                tc.tile_pool(name='psum', bufs=PE_BANKS, space='PSUM'))
            pssm = ctx.enter_context(
                tc.tile_pool(name='psum_s', bufs=1, space='PSUM'))

            constA = cpool.tile([128, 183], F32)
            constB = cpool.tile([16, 2305], F32)
            zsc = cpool.tile([128, 384], BF16)

            # ---- t=0: act-table preload + PE HAM warmup (no DMA deps)
            nc.gpsimd.memset(zsc[:], 0)
            tpre = smpool.tile([128, 1], F32, tag='tpre', name='tpre')
            for af in (AF.Copy, AF.Square, AF.Sqrt, AF.Sigmoid, AF.Relu):
                nc.scalar.activation(tpre[:], zsc[:, 0:1], af)
            wps = pssm.tile([128, 384], F32, tag='dyn', name='warmps')
            for i in range(40):
                nc.tensor.matmul(wps[:], lhsT=zsc[:, 0:128], rhs=zsc[:],
                                 start=(i == 0), stop=(i == 39),
                                 skip_group_check=True)

            # ---- const DMAs FIRST so they beat the x flood ----
            nc.sync.dma_start(constA[:], ca_d[:])
            nc.sync.dma_start(constB[:], cb_d[:])

            def load_tile(s, h):
                """memset guards + 4 quarter DMAs; bn_stats emitted by
                caller via the returned closure."""
                QL = NPIX // 4
                xt = xpool.tile([128, XT_LEN], BF16, tag='x',
                                name=f'x{s}{h}')
                nc.gpsimd.memset(xt[:, 0:DATA_OFF], 0)
                nc.gpsimd.memset(xt[:, DATA_OFF + NPIX:XT_LEN], 0)
                src = x_d[s, 128 * h:128 * (h + 1)].rearrange(
                    'c a b -> c (a b)')
                dmas = []
                for q in range(4):
                    lo = DATA_OFF + QL * q

                    def dma(lo=lo, q=q):
                        nc.sync.dma_start(xt[:, lo:lo + QL],
                                          src[:, QL * q:QL * (q + 1)])
                    dmas.append(dma)
                return xt, dmas

            def stats_dve(xt, tag):
                """DVE bn_stats chain -> [128,2] (mean, pop var)."""
                bn6 = bnpool.tile([128, 24, 6], F32, tag='bn6',
                                  name=f'bn6{tag}')
                for g in range(24):
                    lo = DATA_OFF + 384 * g
                    nc.vector.bn_stats(bn6[:, g, :], xt[:, lo:lo + 384])
                mv = bnpool.tile([128, 2], F32, tag='mv', name=f'mv{tag}')
                nc.vector.bn_aggr(mv[:], bn6[:])
                return mv

            def stats_scalar(xt, tag):
                """ScalarE Copy/Square + accum as 8 emit-on-demand ops."""
                EL = NPIX // 8
                sacc = bnpool.tile([128, 8], F32, tag='sacc',
                                   name=f'sacc{tag}')
                qacc = bnpool.tile([128, 8], F32, tag='qacc',
                                   name=f'qacc{tag}')
                ops = []
                for q in range(8):
                    lo = DATA_OFF + EL * q

                    def op_copy(q=q, lo=lo):
                        dump = dumppool.tile([128, EL], BF16, tag='dump',
                                             name='dump')
                        nc.scalar.activation(dump[:], xt[:, lo:lo + EL],
                                             AF.Copy,
                                             accum_out=sacc[:, q:q + 1])

                    def op_sq(q=q, lo=lo):
                        dump = dumppool.tile([128, EL], BF16, tag='dump',
                                             name='dump')
                        nc.scalar.activation(dump[:], xt[:, lo:lo + EL],
                                             AF.Square,
                                             accum_out=qacc[:, q:q + 1])

                    ops += [op_copy, op_sq]

                def finalize():
                    sq = bnpool.tile([128, 2], F32, tag='sq',
                                     name=f'sq{tag}')
                    nc.vector.tensor_reduce(sq[:, 0:1], sacc[:],
                                            axis=mybir.AxisListType.X,
                                            op=OP.add)
                    nc.vector.tensor_reduce(sq[:, 1:2], qacc[:],
                                            axis=mybir.AxisListType.X,
                                            op=OP.add)
                    mv = bnpool.tile([128, 2], F32, tag='mv',
                                     name=f'mvs{tag}')
                    mom = bnpool.tile([128, 2], F32, tag='mom',
                                      name=f'mom{tag}')
                    nc.vector.tensor_scalar_mul(mom[:], sq[:], 1.0 / NPIX)
                    nc.vector.tensor_copy(mv[:, 0:1], mom[:, 0:1])
                    nc.vector.scalar_tensor_tensor(
                        mv[:, 1:2], mom[:, 0:1], mom[:, 0:1], mom[:, 1:2],
                        OP.mult, OP.subtract)
                    nc.vector.tensor_scalar_mul(mv[:, 1:2], mv[:, 1:2],
                                                -1.0)
                    return mv

                return ops, finalize

            def sample_weights(s, mvs):
                """mvs: dict h -> [128,2] (mean, pop var). Returns per-half
                (weff16 BF16 [128,9], negw16 BF16 [128,9], dg BF16)."""
                g_cols = {}
                gap = smpool.tile([128, 2], F32, tag='gap', name=f'gap{s}')
                for h in (0, 1):
                    m = mvs[h]
                    mean = m[:, 0:1]
                    std = smpool.tile([128, 1], F32, tag='std',
                                      name=f'std{s}{h}')
                    nc.scalar.activation(std[:], m[:, 1:2], AF.Sqrt,
                                         bias=constA[:, 182:183],
                                         scale=float(NPIX) / (NPIX - 1))
                    zt = smpool.tile([128, 1], F32, tag='zt',
                                     name=f'zt{s}{h}')
                    nc.vector.tensor_tensor(zt[:], std[:],
                                            constA[:, 162 + h:163 + h],
                                            OP.mult)
                    nc.vector.scalar_tensor_tensor(
                        zt[:], mean, constA[:, 160 + h:161 + h], zt[:],
                        OP.mult, OP.add)
                    g = smpool.tile([128, 1], F32, tag='g',
                                    name=f'g{s}{h}')
                    nc.scalar.activation(g[:], zt[:], AF.Sigmoid)
                    g_cols[h] = g
                    nc.vector.tensor_tensor(gap[:, h:h + 1], g[:], mean,
                                            OP.mult)
                p = pssm.tile([16, 2], F32, tag='hid', name=f'hid{s}')
                for h in (0, 1):
                    nc.tensor.matmul(p[:, h:h + 1],
                                     lhsT=constA[:, 128 + 16 * h:
                                                 144 + 16 * h],
                                     rhs=gap[:, h:h + 1],
                                     start=True, stop=True)
                hsum = smpool.tile([16, 1], F32, tag='hsum',
                                   name=f'hsum{s}')
                nc.vector.tensor_reduce(hsum[:], p[:],
                                        axis=mybir.AxisListType.X,
                                        op=OP.add)
                hid = smpool.tile([16, 1], F32, tag='hid_sb',
                                  name=f'hidsb{s}')
                nc.scalar.activation(hid[:], hsum[:], AF.Relu,
                                     bias=constB[:, 2304:2305], scale=1.0)
                per_half = {}
                for h in (0, 1):
                    pd = pssm.tile([128, 9], F32, tag='dyn',
                                   name=f'dyn{s}{h}')
                    for k in range(9):
                        nc.tensor.matmul(
                            pd[:, k:k + 1],
                            lhsT=constB[:, k * 256 + 128 * h:
                                        k * 256 + 128 * h + 128],
                            rhs=hid[:], start=True, stop=True)
                    b2g = smpool.tile([128, 9], F32, tag='b2g',
                                      name=f'b2g{s}{h}')
                    nc.vector.tensor_scalar_mul(
                        b2g[:], constA[:, 164 + 9 * h:173 + 9 * h],
                        g_cols[h][:])
                    weff = smpool.tile([128, 9], F32, tag='weff',
                                       name=f'weff{s}{h}')
                    nc.vector.scalar_tensor_tensor(
                        weff[:], pd[:], g_cols[h][:], b2g[:], OP.mult,
                        OP.add)
                    weff16 = smpool.tile([128, 9], BF16, tag='weff16',
                                         name=f'weff16{s}{h}')
                    nc.vector.tensor_copy(weff16[:], weff[:])
                    negw16 = smpool.tile([128, 9], BF16, tag='negw16',
                                         name=f'negw16{s}{h}')
                    nc.vector.tensor_scalar_mul(negw16[:], weff[:], -1.0)
                    dg = dgpool.tile([128, 9 * 128], BF16, tag='diag',
                                     name=f'diag{s}{h}')
                    for k in range(9):
                        nc.scalar.activation(
                            dg[:, 128 * k:128 * (k + 1)],
                            constA[:, 0:128], AF.Copy,
                            scale=weff[:, k:k + 1])
                    per_half[h] = (weff, weff16, negw16, dg)
                return per_half

            def conv_tile(s, h, xt, weff, weff16, negw16, dg, fillers=None,
                          last=False):
                """Emit full conv for one tile. fillers: list of 0-arg
                closures drained between PE rounds (ScalarE stats ops)."""
                fillers = list(fillers or [])
                xfr = xt[:].rearrange('p (r c) -> p r c', c=W)
                out_flat = out_d[s, 128 * h:128 * (h + 1)].rearrange(
                    'c a b -> c (a b)')

                def take_fillers(k):
                    for _ in range(min(k, len(fillers))):
                        fillers.pop(0)()

                # ---- PE rows [0, P_PE), taps-outer over PE_BANKS banks
                rounds = []
                rs = 0
                while rs < P_PE:
                    re = min(rs + PE_BANKS * PE_CHUNK, P_PE)
                    rounds.append((rs, re))
                    rs = re
                for (ra, rb) in rounds:
                    nchunk = (rb - ra) // PE_CHUNK
                    ps = [pspool.tile([128, PE_CHUNK * W], F32, tag='cps',
                                      name=f'cps{s}{h}{ra}_{b}')
                          for b in range(nchunk)]
                    for k in range(9):
                        dy, dx = _tap(k)
                        for b in range(nchunk):
                            rc = ra + b * PE_CHUNK
                            off = DATA_OFF + (rc + dy) * W + dx
                            nc.tensor.matmul(
                                ps[b][:],
                                lhsT=dg[:, 128 * k:128 * (k + 1)],
                                rhs=xt[:, off:off + PE_CHUNK * W],
                                start=(k == 0), stop=(k == 8),
                                skip_group_check=True)
                    st = stpe.tile([128, (rb - ra) * W], BF16, tag='st_pe',
                                   name=f'stpe{s}{h}{ra}')
                    str_ = st[:].rearrange('p (r c) -> p r c', c=W)
                    for b in range(nchunk):
                        nc.scalar.copy(
                            st[:, b * PE_CHUNK * W:
                               (b + 1) * PE_CHUNK * W], ps[b][:])
                        if b % 2 == 0:
                            take_fillers(1)
                    nrow = rb - ra
                    # wrap fixups on DVE (flat-read contamination)
                    for dy in (-1, 0, 1):
                        kp = (dy + 1) * 3 + 2
                        nc.vector.scalar_tensor_tensor(
                            str_[:, 0:nrow, W - 1:W],
                            xfr[:, ra + dy + GUARD_TOP + 1:
                                rb + dy + GUARD_TOP + 1, 0:1],
                            negw16[:, kp:kp + 1],
                            str_[:, 0:nrow, W - 1:W],
                            OP.mult, OP.add)
                        km = (dy + 1) * 3
                        nc.vector.scalar_tensor_tensor(
                            str_[:, 0:nrow, 0:1],
                            xfr[:, ra + dy + GUARD_TOP - 1:
                                rb + dy + GUARD_TOP - 1, W - 1:W],
                            negw16[:, km:km + 1],
                            str_[:, 0:nrow, 0:1],
                            OP.mult, OP.add)
                    nc.sync.dma_start(out_flat[:, ra * W:rb * W], st[:])

                # ---- engine rows: DVE [P_PE, +P_DVE), GP [.., +P_GP)
                specs = [('dve', nc.vector, P_PE, P_PE + P_DVE, stdve)]
                if P_GP:
                    specs.append(('gp', nc.gpsimd, P_PE + P_DVE, H, stgp))
                for (nm, eng, a, b, pool) in specs:
                    nrow = b - a
                    st = pool.tile([128, nrow * W], BF16, tag=f'st_{nm}',
                                   name=f'st{nm}{s}{h}')
                    str_ = st[:].rearrange('p (r c) -> p r c', c=W)
                    for k in (4, 0, 1, 2, 3, 5, 6, 7, 8):
                        dy, dx = _tap(k)
                        if dx == 1:
                            co0, co1 = 0, W - 1
                        elif dx == -1:
                            co0, co1 = 1, W
                        else:
                            co0, co1 = 0, W
                        o_ap = str_[:, 0:nrow, co0:co1]
                        i_ap = xfr[:, a + dy + GUARD_TOP:
                                   b + dy + GUARD_TOP,
                                   co0 + dx:co1 + dx]
                        if k == 4:
                            nc.scalar.activation(o_ap, i_ap, AF.Copy,
                                                 scale=weff[:, 4:5])
                            take_fillers(1)
                        else:
                            eng.scalar_tensor_tensor(
                                o_ap, i_ap, weff16[:, k:k + 1], o_ap,
                                OP.mult, OP.add)
                    if last and nm == 'dve':
                        mid = a + nrow // 2
                        nc.sync.dma_start(out_flat[:, a * W:mid * W],
                                          st[:, 0:(mid - a) * W])
                        nc.sync.dma_start(out_flat[:, mid * W:b * W],
                                          st[:, (mid - a) * W:])
                    else:
                        nc.sync.dma_start(out_flat[:, a * W:b * W], st[:])
                for f in fillers:
                    f()

            # ================= pipeline =================
            xt00, d00 = load_tile(0, 0)
            xt01, d01 = load_tile(0, 1)
            for q in range(4):
                d00[q]()
                d01[q]()
            xt10, d10 = load_tile(1, 0)
            xt11, d11 = load_tile(1, 1)
            for q in range(4):
                d10[q]()
                d11[q]()
            mv00 = stats_dve(xt00, '00')
            ops01, fin01 = stats_scalar(xt01, '01')
            for op in ops01:
                op()
            mv01 = fin01()
            ph0 = sample_weights(0, {0: mv00, 1: mv01})
            ops11, fin11 = stats_scalar(xt11, '11')
            conv_tile(0, 0, xt00, *ph0[0], fillers=ops11)
            mv10 = stats_dve(xt10, '10')
            conv_tile(0, 1, xt01, *ph0[1])
            mv11 = fin11()
            ph1 = sample_weights(1, {0: mv10, 1: mv11})
            conv_tile(1, 0, xt10, *ph1[0])
            conv_tile(1, 1, xt11, *ph1[1], last=True)

    nc.compile()
    return nc


def _host_constants(cfc, w1, b1, w2, b2):
    A = np.zeros((128, 183), np.float32)
    A[:, 0:128] = np.eye(128, dtype=np.float32)
    w1T = np.ascontiguousarray(w1.T)              # [256, 16]
    A[:, 128:144] = w1T[:128]
    A[:, 144:160] = w1T[128:]
    A[:, 160] = cfc[0:128, 0]
    A[:, 161] = cfc[128:256, 0]
    A[:, 162] = cfc[0:128, 1]
    A[:, 163] = cfc[128:256, 1]
    b2r = b2.reshape(256, 9)
    A[:, 164:173] = b2r[0:128]
    A[:, 173:182] = b2r[128:256]
    A[:, 182] = EPS
    w2p = w2.reshape(256, 9, 16).transpose(1, 0, 2).reshape(2304, 16)
    B = np.zeros((16, 2305), np.float32)
    B[:, 0:2304] = w2p.T
    B[:, 2304] = b1
    return A, B


def kernel(x, cfc, w1, b1, w2, b2):
    global LAST_EXEC_NS, LAST_RESULTS
    _install_trace_hook_shim()
    from concourse.bass_utils import run_bass_kernel_spmd

    x16 = np.ascontiguousarray(np.asarray(x, np.float32)).astype(BF)
    A, B = _host_constants(np.asarray(cfc, np.float32),
                           np.asarray(w1, np.float32),
                           np.asarray(b1, np.float32),
                           np.asarray(w2, np.float32),
                           np.asarray(b2, np.float32))

    if 'nc' not in _PROGRAM_CACHE:
        _PROGRAM_CACHE['nc'] = _build_program()
    nc = _PROGRAM_CACHE['nc']

    in_maps = [{'x': x16[S_PER_CORE * i:S_PER_CORE * (i + 1)],
                'constA': A, 'constB': B} for i in range(CORES)]
    res = run_bass_kernel_spmd(nc, in_maps, list(range(CORES)))
    LAST_EXEC_NS = res.exec_time_ns
    LAST_RESULTS = res
    out = np.concatenate([res.results[i]['out'] for i in range(CORES)],
                         axis=0)
    return out.astype(np.float32)


# revision 7
# speedup vs baseline: 1.0506x; 1.0079x over previous
"""Trainium2 Bass kernel for nn_DCSRM — bf16 rewrite.

Same math as v1 (g folds into the 9 tap weights; srm never materialized)
but all image traffic is bf16: host converts x fp32->bf16, kernel writes
bf16 output, host converts back. Halves HBM bytes both ways.

Per core (2 samples x 2 channel-halves = 4 tiles [128ch, 99*96] bf16):
  stats:  DVE bn_stats for tiles h=0, ScalarE Copy/Square+accum for h=1
  conv:   rows split PE (diag-bf16 matmuls, taps-outer over 6 psum banks)
          / DVE (8-tap STT FMA, center tap on ScalarE)
          / GpSimd (8-tap STT FMA)
  drains: psum -> bf16 stage on ScalarE/GpSimd, wrap fixups on DVE
"""
import os
import sys
import types
import contextlib
from contextlib import ExitStack

sys.path.insert(0, '/opt/trn_rl_repo')

import numpy as np
import ml_dtypes

BF = np.dtype(ml_dtypes.bfloat16)

N, C, H, W = 16, 256, 96, 96
EPS = 1e-5
NPIX = H * W                      # 9216
CORES = 8
S_PER_CORE = N // CORES           # 2 samples per core

GUARD_TOP = 2
GUARD_BOT = 1
ROWS_BUF = GUARD_TOP + H + GUARD_BOT          # 99
XT_LEN = ROWS_BUF * W                          # 9504
DATA_OFF = GUARD_TOP * W                       # 192

# conv row ownership per tile: (pe_rows, dve_rows, gp_rows), sum = 96
ROW_SPLIT = (68, 28, 0)
PE_CHUNK = 4                      # rows per psum bank (4*96=384 <= 512)
PE_BANKS = 6                      # conv psum banks; 2 left for weights

LAST_EXEC_NS = None
LAST_RESULTS = None
_PROGRAM_CACHE = {}


def _install_trace_hook_shim():
    """Register the NTFF hook via ctypes if antenv.axon_hooks is absent."""
    try:
        import antenv.axon_hooks  # noqa: F401
        return
    except ImportError:
        pass
    try:
        import antenv
        import ctypes
    except ImportError:
        return
    so_path = '/opt/axon/libaxon_pjrt.so'

    def _build():
        if not os.path.exists(so_path):
            return None
        lib = ctypes.CDLL(so_path)
        if not hasattr(lib, 'axon_start_nrt_profile'):
            return None
        lib.axon_start_nrt_profile.argtypes = [
            ctypes.POINTER(ctypes.c_int64), ctypes.c_size_t]
        lib.axon_start_nrt_profile.restype = ctypes.c_int64
        lib.axon_stop_nrt_profile.argtypes = [ctypes.c_char_p]
        lib.axon_stop_nrt_profile.restype = ctypes.c_int64

        @contextlib.contextmanager
        def _hook(output_dir, device_ids):
            import jax
            jax.devices()
            if device_ids:
                ids = (ctypes.c_int64 * len(device_ids))(*device_ids)
                rc = lib.axon_start_nrt_profile(ids, len(device_ids))
            else:
                rc = lib.axon_start_nrt_profile(None, 0)
            if rc != 0:
                raise RuntimeError(f'axon_start_nrt_profile rc={rc}')
            try:
                yield
            finally:
                n = lib.axon_stop_nrt_profile(str(output_dir).encode())
                print(f'ntff profile: {n} file(s) -> {output_dir}',
                      file=sys.stderr)
        return _hook

    mod = types.ModuleType('antenv.axon_hooks')
    holder = {'hook': _build()}
    mod.get_axon_ntff_profile_hook = lambda: holder['hook']
    mod.set_axon_ntff_profile_hook = lambda h: holder.update(hook=h)
    sys.modules['antenv.axon_hooks'] = mod
    antenv.axon_hooks = mod


def _tap(k):
    return k // 3 - 1, k % 3 - 1          # dy, dx


def _build_program():
    from concourse import bacc, mybir, tile

    F32 = mybir.dt.float32
    BF16 = mybir.dt.bfloat16
    OP = mybir.AluOpType
    AF = mybir.ActivationFunctionType

    nc = bacc.Bacc('TRN2', target_bir_lowering=False, debug=False,
                   num_devices=CORES)

    x_d = nc.dram_tensor('x', [S_PER_CORE, C, H, W], BF16,
                         kind='ExternalInput').ap()
    ca_d = nc.dram_tensor('constA', [128, 183], F32,
                          kind='ExternalInput').ap()
    cb_d = nc.dram_tensor('constB', [16, 2305], F32,
                          kind='ExternalInput').ap()
    out_d = nc.dram_tensor('out', [S_PER_CORE, C, H, W], BF16,
                           kind='ExternalOutput').ap()

    P_PE, P_DVE, P_GP = ROW_SPLIT
    assert P_PE + P_DVE + P_GP == H
    assert P_PE % PE_CHUNK == 0

    with tile.TileContext(nc) as tc:
        with ExitStack() as ctx:
            cpool = ctx.enter_context(tc.tile_pool(name='const', bufs=1))
            xpool = ctx.enter_context(tc.tile_pool(name='x', bufs=4))
            bnpool = ctx.enter_context(tc.tile_pool(name='bn', bufs=4))
            dumppool = ctx.enter_context(tc.tile_pool(name='dump', bufs=1))
            smpool = ctx.enter_context(tc.tile_pool(name='small', bufs=4))
            dgpool = ctx.enter_context(tc.tile_pool(name='diag', bufs=2))
            stpe = ctx.enter_context(tc.tile_pool(name='stage_pe', bufs=3))
            stdve = ctx.enter_context(tc.tile_pool(name='stage_dve', bufs=2))
            stgp = ctx.enter_context(tc.tile_pool(name='stage_gp', bufs=2))
            pspool = ctx.enter_context(
                tc.tile_pool(name='psum', bufs=PE_BANKS, space='PSUM'))
            pssm = ctx.enter_context(
                tc.tile_pool(name='psum_s', bufs=1, space='PSUM'))

            constA = cpool.tile([128, 183], F32)
            constB = cpool.tile([16, 2305], F32)
            zsc = cpool.tile([128, 384], BF16)

            # ---- t=0: act-table preload + PE HAM warmup (no DMA deps)
            nc.gpsimd.memset(zsc[:], 0)
            tpre = smpool.tile([128, 1], F32, tag='tpre', name='tpre')
            for af in (AF.Copy, AF.Square, AF.Sqrt, AF.Sigmoid, AF.Relu):
                nc.scalar.activation(tpre[:], zsc[:, 0:1], af)
            wps = pssm.tile([128, 384], F32, tag='dyn', name='warmps')
            for i in range(40):
                nc.tensor.matmul(wps[:], lhsT=zsc[:, 0:128], rhs=zsc[:],
                                 start=(i == 0), stop=(i == 39),
                                 skip_group_check=True)

            # ---- const DMAs FIRST so they beat the x flood ----
            nc.sync.dma_start(constA[:], ca_d[:])
            nc.sync.dma_start(constB[:], cb_d[:])

            def load_tile(s, h):
                """memset guards + 4 quarter DMAs; bn_stats emitted by
                caller via the returned closure."""
                QL = NPIX // 4
                xt = xpool.tile([128, XT_LEN], BF16, tag='x',
                                name=f'x{s}{h}')
                nc.gpsimd.memset(xt[:, 0:DATA_OFF], 0)
                nc.gpsimd.memset(xt[:, DATA_OFF + NPIX:XT_LEN], 0)
                src = x_d[s, 128 * h:128 * (h + 1)].rearrange(
                    'c a b -> c (a b)')
                dmas = []
                for q in range(4):
                    lo = DATA_OFF + QL * q

                    def dma(lo=lo, q=q):
                        nc.sync.dma_start(xt[:, lo:lo + QL],
                                          src[:, QL * q:QL * (q + 1)])
                    dmas.append(dma)
                return xt, dmas

            def stats_dve(xt, tag):
                """DVE bn_stats chain -> [128,2] (mean, pop var)."""
                bn6 = bnpool.tile([128, 24, 6], F32, tag='bn6',
                                  name=f'bn6{tag}')
                for g in range(24):
                    lo = DATA_OFF + 384 * g
                    nc.vector.bn_stats(bn6[:, g, :], xt[:, lo:lo + 384])
                mv = bnpool.tile([128, 2], F32, tag='mv', name=f'mv{tag}')
                nc.vector.bn_aggr(mv[:], bn6[:])
                return mv

            def stats_scalar(xt, tag):
                """ScalarE Copy/Square + accum as 8 emit-on-demand ops."""
                EL = NPIX // 8
                sacc = bnpool.tile([128, 8], F32, tag='sacc',
                                   name=f'sacc{tag}')
                qacc = bnpool.tile([128, 8], F32, tag='qacc',
                                   name=f'qacc{tag}')
                ops = []
                for q in range(8):
                    lo = DATA_OFF + EL * q

                    def op_copy(q=q, lo=lo):
                        dump = dumppool.tile([128, EL], BF16, tag='dump',
                                             name='dump')
                        nc.scalar.activation(dump[:], xt[:, lo:lo + EL],
                                             AF.Copy,
                                             accum_out=sacc[:, q:q + 1])

                    def op_sq(q=q, lo=lo):
                        dump = dumppool.tile([128, EL], BF16, tag='dump',
                                             name='dump')
                        nc.scalar.activation(dump[:], xt[:, lo:lo + EL],
                                             AF.Square,
                                             accum_out=qacc[:, q:q + 1])

                    ops += [op_copy, op_sq]

                def finalize():
                    sq = bnpool.tile([128, 2], F32, tag='sq',
                                     name=f'sq{tag}')
                    nc.vector.tensor_reduce(sq[:, 0:1], sacc[:],
                                            axis=mybir.AxisListType.X,
                                            op=OP.add)
                    nc.vector.tensor_reduce(sq[:, 1:2], qacc[:],
                                            axis=mybir.AxisListType.X,
                                            op=OP.add)
                    mv = bnpool.tile([128, 2], F32, tag='mv',
                                     name=f'mvs{tag}')
                    mom = bnpool.tile([128, 2], F32, tag='mom',
                                      name=f'mom{tag}')
                    nc.vector.tensor_scalar_mul(mom[:], sq[:], 1.0 / NPIX)
                    nc.vector.tensor_copy(mv[:, 0:1], mom[:, 0:1])
                    nc.vector.scalar_tensor_tensor(
                        mv[:, 1:2], mom[:, 0:1], mom[:, 0:1], mom[:, 1:2],
                        OP.mult, OP.subtract)
                    nc.vector.tensor_scalar_mul(mv[:, 1:2], mv[:, 1:2],
                                                -1.0)
                    return mv

                return ops, finalize

            def stats_split(xt, tag):
                EL = NPIX // 8
                sacc = bnpool.tile([128, 4], F32, tag='sacc',
                                   name=f'sacH{tag}')
                qacc = bnpool.tile([128, 4], F32, tag='qacc',
                                   name=f'qacH{tag}')
                ops = []
                for q in range(4):
                    lo = DATA_OFF + EL * q

                    def op_copy(q=q, lo=lo):
                        dump = dumppool.tile([128, EL], BF16, tag='dump',
                                             name='dump')
                        nc.scalar.activation(dump[:], xt[:, lo:lo + EL],
                                             AF.Copy,
                                             accum_out=sacc[:, q:q + 1])

                    def op_sq(q=q, lo=lo):
                        dump = dumppool.tile([128, EL], BF16, tag='dump',
                                             name='dump')
                        nc.scalar.activation(dump[:], xt[:, lo:lo + EL],
                                             AF.Square,
                                             accum_out=qacc[:, q:q + 1])

                    ops += [op_copy, op_sq]

                def dve_part():
                    bnh = bnpool.tile([128, 12, 6], F32, tag='bnh',
                                      name=f'bnh{tag}')
                    for g in range(12, 24):
                        lo = DATA_OFF + 384 * g
                        nc.vector.bn_stats(bnh[:, g - 12, :],
                                           xt[:, lo:lo + 384])
                    mvh = bnpool.tile([128, 2], F32, tag='mvh',
                                      name=f'mvh{tag}')
                    nc.vector.bn_aggr(mvh[:], bnh[:])
                    return mvh

                def finalize(mvh):
                    HN = float(NPIX // 2)
                    tot = bnpool.tile([128, 2], F32, tag='tot',
                                      name=f'tot{tag}')
                    nc.vector.tensor_reduce(tot[:, 0:1], sacc[:],
                                            axis=mybir.AxisListType.X,
                                            op=OP.add)
                    nc.vector.tensor_reduce(tot[:, 1:2], qacc[:],
                                            axis=mybir.AxisListType.X,
                                            op=OP.add)
                    s2 = bnpool.tile([128, 2], F32, tag='s2',
                                     name=f's2{tag}')
                    nc.vector.tensor_scalar_mul(s2[:, 0:1], mvh[:, 0:1],
                                                HN)
                    nc.vector.scalar_tensor_tensor(
                        s2[:, 1:2], mvh[:, 0:1], mvh[:, 0:1],
                        mvh[:, 1:2], OP.mult, OP.add)
                    nc.vector.tensor_scalar_mul(s2[:, 1:2], s2[:, 1:2],
                                                HN)
                    nc.vector.tensor_tensor(tot[:], tot[:], s2[:],
                                            OP.add)
                    mv = bnpool.tile([128, 2], F32, tag='mv',
                                     name=f'mvS{tag}')
                    mom = bnpool.tile([128, 2], F32, tag='mom',
                                      name=f'momS{tag}')
                    nc.vector.tensor_scalar_mul(mom[:], tot[:],
                                                1.0 / NPIX)
                    nc.vector.tensor_copy(mv[:, 0:1], mom[:, 0:1])
                    nc.vector.scalar_tensor_tensor(
                        mv[:, 1:2], mom[:, 0:1], mom[:, 0:1],
                        mom[:, 1:2], OP.mult, OP.subtract)
                    nc.vector.tensor_scalar_mul(mv[:, 1:2], mv[:, 1:2],
                                                -1.0)
                    return mv

                return ops, dve_part, finalize

            def sample_weights(s, mvs):
                """mvs: dict h -> [128,2] (mean, pop var). Returns per-half
                (weff16 BF16 [128,9], negw16 BF16 [128,9], dg BF16)."""
                g_cols = {}
                gap = smpool.tile([128, 2], F32, tag='gap', name=f'gap{s}')
                for h in (0, 1):
                    m = mvs[h]
                    mean = m[:, 0:1]
                    std = smpool.tile([128, 1], F32, tag='std',
                                      name=f'std{s}{h}')
                    nc.scalar.activation(std[:], m[:, 1:2], AF.Sqrt,
                                         bias=constA[:, 182:183],
                                         scale=float(NPIX) / (NPIX - 1))
                    zt = smpool.tile([128, 1], F32, tag='zt',
                                     name=f'zt{s}{h}')
                    nc.vector.tensor_tensor(zt[:], std[:],
                                            constA[:, 162 + h:163 + h],
                                            OP.mult)
                    nc.vector.scalar_tensor_tensor(
                        zt[:], mean, constA[:, 160 + h:161 + h], zt[:],
                        OP.mult, OP.add)
                    g = smpool.tile([128, 1], F32, tag='g',
                                    name=f'g{s}{h}')
                    nc.scalar.activation(g[:], zt[:], AF.Sigmoid)
                    g_cols[h] = g
                    nc.vector.tensor_tensor(gap[:, h:h + 1], g[:], mean,
                                            OP.mult)
                p = pssm.tile([16, 2], F32, tag='hid', name=f'hid{s}')
                for h in (0, 1):
                    nc.tensor.matmul(p[:, h:h + 1],
                                     lhsT=constA[:, 128 + 16 * h:
                                                 144 + 16 * h],
                                     rhs=gap[:, h:h + 1],
                                     start=True, stop=True)
                hsum = smpool.tile([16, 1], F32, tag='hsum',
                                   name=f'hsum{s}')
                nc.vector.tensor_reduce(hsum[:], p[:],
                                        axis=mybir.AxisListType.X,
                                        op=OP.add)
                hid = smpool.tile([16, 1], F32, tag='hid_sb',
                                  name=f'hidsb{s}')
                nc.scalar.activation(hid[:], hsum[:], AF.Relu,
                                     bias=constB[:, 2304:2305], scale=1.0)
                per_half = {}
                for h in (0, 1):
                    pd = pssm.tile([128, 9], F32, tag='dyn',
                                   name=f'dyn{s}{h}')
                    for k in range(9):
                        nc.tensor.matmul(
                            pd[:, k:k + 1],
                            lhsT=constB[:, k * 256 + 128 * h:
                                        k * 256 + 128 * h + 128],
                            rhs=hid[:], start=True, stop=True)
                    b2g = smpool.tile([128, 9], F32, tag='b2g',
                                      name=f'b2g{s}{h}')
                    nc.vector.tensor_scalar_mul(
                        b2g[:], constA[:, 164 + 9 * h:173 + 9 * h],
                        g_cols[h][:])
                    weff = smpool.tile([128, 9], F32, tag='weff',
                                       name=f'weff{s}{h}')
                    nc.vector.scalar_tensor_tensor(
                        weff[:], pd[:], g_cols[h][:], b2g[:], OP.mult,
                        OP.add)
                    weff16 = smpool.tile([128, 9], BF16, tag='weff16',
                                         name=f'weff16{s}{h}')
                    nc.vector.tensor_copy(weff16[:], weff[:])
                    negw16 = smpool.tile([128, 9], BF16, tag='negw16',
                                         name=f'negw16{s}{h}')
                    nc.vector.tensor_scalar_mul(negw16[:], weff[:], -1.0)
                    dg = dgpool.tile([128, 9 * 128], BF16, tag='diag',
                                     name=f'diag{s}{h}')
                    for k in range(9):
                        nc.scalar.activation(
                            dg[:, 128 * k:128 * (k + 1)],
                            constA[:, 0:128], AF.Copy,
                            scale=weff[:, k:k + 1])
                    per_half[h] = (weff, weff16, negw16, dg)
                return per_half

            def conv_tile(s, h, xt, weff, weff16, negw16, dg, fillers=None,
                          last=False):
                """Emit full conv for one tile. fillers: list of 0-arg
                closures drained between PE rounds (ScalarE stats ops)."""
                fillers = list(fillers or [])
                xfr = xt[:].rearrange('p (r c) -> p r c', c=W)
                out_flat = out_d[s, 128 * h:128 * (h + 1)].rearrange(
                    'c a b -> c (a b)')

                def take_fillers(k):
                    for _ in range(min(k, len(fillers))):
                        fillers.pop(0)()

                # ---- PE rows [0, P_PE), taps-outer over PE_BANKS banks
                rounds = []
                rs = 0
                while rs < P_PE:
                    re = min(rs + PE_BANKS * PE_CHUNK, P_PE)
                    rounds.append((rs, re))
                    rs = re
                for (ra, rb) in rounds:
                    nchunk = (rb - ra) // PE_CHUNK
                    ps = [pspool.tile([128, PE_CHUNK * W], F32, tag='cps',
                                      name=f'cps{s}{h}{ra}_{b}')
                          for b in range(nchunk)]
                    for k in range(9):
                        dy, dx = _tap(k)
                        for b in range(nchunk):
                            rc = ra + b * PE_CHUNK
                            off = DATA_OFF + (rc + dy) * W + dx
                            nc.tensor.matmul(
                                ps[b][:],
                                lhsT=dg[:, 128 * k:128 * (k + 1)],
                                rhs=xt[:, off:off + PE_CHUNK * W],
                                start=(k == 0), stop=(k == 8),
                                skip_group_check=True)
                    st = stpe.tile([128, (rb - ra) * W], BF16, tag='st_pe',
                                   name=f'stpe{s}{h}{ra}')
                    str_ = st[:].rearrange('p (r c) -> p r c', c=W)
                    for b in range(nchunk):
                        nc.scalar.copy(
                            st[:, b * PE_CHUNK * W:
                               (b + 1) * PE_CHUNK * W], ps[b][:])
                        take_fillers(1)
                    nrow = rb - ra
                    # wrap fixups on DVE (flat-read contamination)
                    for dy in (-1, 0, 1):
                        kp = (dy + 1) * 3 + 2
                        nc.vector.scalar_tensor_tensor(
                            str_[:, 0:nrow, W - 1:W],
                            xfr[:, ra + dy + GUARD_TOP + 1:
                                rb + dy + GUARD_TOP + 1, 0:1],
                            negw16[:, kp:kp + 1],
                            str_[:, 0:nrow, W - 1:W],
                            OP.mult, OP.add)
                        km = (dy + 1) * 3
                        nc.vector.scalar_tensor_tensor(
                            str_[:, 0:nrow, 0:1],
                            xfr[:, ra + dy + GUARD_TOP - 1:
                                rb + dy + GUARD_TOP - 1, W - 1:W],
                            negw16[:, km:km + 1],
                            str_[:, 0:nrow, 0:1],
                            OP.mult, OP.add)
                    nc.sync.dma_start(out_flat[:, ra * W:rb * W], st[:])

                # ---- engine rows: DVE [P_PE, +P_DVE), GP [.., +P_GP)
                specs = [('dve', nc.vector, P_PE, P_PE + P_DVE, stdve)]
                if P_GP:
                    specs.append(('gp', nc.gpsimd, P_PE + P_DVE, H, stgp))
                for (nm, eng, a, b, pool) in specs:
                    nrow = b - a
                    st = pool.tile([128, nrow * W], BF16, tag=f'st_{nm}',
                                   name=f'st{nm}{s}{h}')
                    str_ = st[:].rearrange('p (r c) -> p r c', c=W)
                    for k in (4, 0, 1, 2, 3, 5, 6, 7, 8):
                        dy, dx = _tap(k)
                        if dx == 1:
                            co0, co1 = 0, W - 1
                        elif dx == -1:
                            co0, co1 = 1, W
                        else:
                            co0, co1 = 0, W
                        o_ap = str_[:, 0:nrow, co0:co1]
                        i_ap = xfr[:, a + dy + GUARD_TOP:
                                   b + dy + GUARD_TOP,
                                   co0 + dx:co1 + dx]
                        if k == 4:
                            nc.scalar.activation(o_ap, i_ap, AF.Copy,
                                                 scale=weff[:, 4:5])
                            take_fillers(1)
                        else:
                            eng.scalar_tensor_tensor(
                                o_ap, i_ap, weff16[:, k:k + 1], o_ap,
                                OP.mult, OP.add)
                    if last and nm == 'dve':
                        mid = a + nrow // 2
                        nc.sync.dma_start(out_flat[:, a * W:mid * W],
                                          st[:, 0:(mid - a) * W])
                        nc.sync.dma_start(out_flat[:, mid * W:b * W],
                                          st[:, (mid - a) * W:])
                    else:
                        nc.sync.dma_start(out_flat[:, a * W:b * W], st[:])
                for f in fillers:
                    f()

            # ================= pipeline =================
            xt00, d00 = load_tile(0, 0)
            xt01, d01 = load_tile(0, 1)
            for q in range(4):
                d00[q]()
                d01[q]()
            xt10, d10 = load_tile(1, 0)
            xt11, d11 = load_tile(1, 1)
            for q in range(4):
                d10[q]()
                d11[q]()
            mv00 = stats_dve(xt00, '00')
            ops01, dvep01, fin01c = stats_split(xt01, '01')
            for op in ops01:
                op()
            mvh01 = dvep01()
            mv01 = fin01c(mvh01)
            ph0 = sample_weights(0, {0: mv00, 1: mv01})
            ops11, fin11 = stats_scalar(xt11, '11')
            conv_tile(0, 0, xt00, *ph0[0], fillers=ops11)
            mv10 = stats_dve(xt10, '10')
            conv_tile(0, 1, xt01, *ph0[1])
            mv11 = fin11()
            ph1 = sample_weights(1, {0: mv10, 1: mv11})
            conv_tile(1, 0, xt10, *ph1[0])
            conv_tile(1, 1, xt11, *ph1[1], last=True)

    nc.compile()
    return nc


def _host_constants(cfc, w1, b1, w2, b2):
    A = np.zeros((128, 183), np.float32)
    A[:, 0:128] = np.eye(128, dtype=np.float32)
    w1T = np.ascontiguousarray(w1.T)              # [256, 16]
    A[:, 128:144] = w1T[:128]
    A[:, 144:160] = w1T[128:]
    A[:, 160] = cfc[0:128, 0]
    A[:, 161] = cfc[128:256, 0]
    A[:, 162] = cfc[0:128, 1]
    A[:, 163] = cfc[128:256, 1]
    b2r = b2.reshape(256, 9)
    A[:, 164:173] = b2r[0:128]
    A[:, 173:182] = b2r[128:256]
    A[:, 182] = EPS
    w2p = w2.reshape(256, 9, 16).transpose(1, 0, 2).reshape(2304, 16)
    B = np.zeros((16, 2305), np.float32)
    B[:, 0:2304] = w2p.T
    B[:, 2304] = b1
    return A, B


def kernel(x, cfc, w1, b1, w2, b2):
    global LAST_EXEC_NS, LAST_RESULTS
    _install_trace_hook_shim()
    from concourse.bass_utils import run_bass_kernel_spmd

    x16 = np.ascontiguousarray(np.asarray(x, np.float32)).astype(BF)
    A, B = _host_constants(np.asarray(cfc, np.float32),
                           np.asarray(w1, np.float32),
                           np.asarray(b1, np.float32),
                           np.asarray(w2, np.float32),
                           np.asarray(b2, np.float32))

    if 'nc' not in _PROGRAM_CACHE:
        _PROGRAM_CACHE['nc'] = _build_program()
    nc = _PROGRAM_CACHE['nc']

    in_maps = [{'x': x16[S_PER_CORE * i:S_PER_CORE * (i + 1)],
                'constA': A, 'constB': B} for i in range(CORES)]
    res = run_bass_kernel_spmd(nc, in_maps, list(range(CORES)))
    LAST_EXEC_NS = res.exec_time_ns
    LAST_RESULTS = res
    out = np.concatenate([res.results[i]['out'] for i in range(CORES)],
                         axis=0)
    return out.astype(np.float32)
